# revision 25
# baseline (speedup 1.0000x reference)
"""Trainium2 Bass kernel for MACE-style GNN message-passing convolution.

Strategy (8 NeuronCores, full I/O):
  * Host partitions the 16384 nodes into 128 bins (8 cores x 16 chunks) of
    exactly 128 nodes each, balancing in-degree so every bin receives ~1024
    edges (exactly 1024 when the repair pass converges).  Each core owns the
    output rows of its 16 chunks -> no cross-core collective is needed.
  * Edges are routed to the (core, chunk) owning their receiver.  Per chunk
    the kernel gathers sender rows with one dma_gather (HBM -> SBUF, edges
    land on partitions), runs the radial MLP on PE/ACT, builds the weighted
    tensor-product messages on DVE/ACT, and scatter-adds them into a PSUM
    accumulator via one-hot matmuls (receiver one-hot built on-chip with
    iota + is_equal; the es-scaled message groups fold the edge scalar into
    the one-hot).
  * bf16 data path (f32 PSUM accumulation), f32 output.
"""

import sys

sys.path.insert(0, "/opt/trn_rl_repo")

import heapq

import numpy as np

import concourse.bacc as bacc
import concourse.bass as bass
import concourse.mybir as mybir
import concourse.tile as tile
from concourse.bass_utils import run_bass_kernel_spmd

# ---------------------------------------------------------------- constants
N_NODES = 16384
N_EDGES = 131072
N_CORES = 8
N_CHUNKS = 16            # chunks (of 128 output nodes) per core
N_BINS = N_CORES * N_CHUNKS
BIN_NODES = 128
TARGET_LOAD = N_EDGES // N_BINS  # 1024
MUL = 64
N_RADIAL = 8
HIDDEN = 64
INV_SQRT3 = 1.0 / np.sqrt(3.0)

F32 = mybir.dt.float32
BF16 = mybir.dt.bfloat16
I16 = mybir.dt.int16
I32 = mybir.dt.int32
NP_BF16 = mybir.dt.np(BF16)

AF = mybir.ActivationFunctionType
ALU = mybir.AluOpType

# message-slab column layout (64-wide blocks):
#   [g5 g5 g5 | g1 | g0 | g2 | g3 g3 g3 | g4 g4 g4]
# g0 = s*w0, g1 = s*w1 (es via one-hot), g2 = (v.ev)*w2, g3 = v*w3,
# g4 = (s*w4) x ev, g5 = v*w5 (es via one-hot)
# w-slab order: [w5 | w1 | w0 | w2 | w3 | w4]


def _w3_permuted(w3: np.ndarray) -> np.ndarray:
    """Reference w3 [64, 384] -> device slab order with norm factors."""
    scale = 1.0 / (np.sqrt(HIDDEN) * np.sqrt(8.0))  # mlp fan-in + avg-neighbors
    blocks = {
        "w0": w3[:, 0:64],
        "w1": w3[:, 64:128],
        "w2": w3[:, 128:192] * INV_SQRT3,
        "w3": w3[:, 192:256],
        "w4": w3[:, 256:320],
        "w5": w3[:, 320:384],
    }
    out = np.concatenate(
        [blocks["w5"], blocks["w1"], blocks["w0"], blocks["w2"], blocks["w3"],
         blocks["w4"]], axis=1)
    return out * scale


def _ref_colmap() -> np.ndarray:
    """refcol[d] = reference output column for device column d."""
    refcol = np.empty(768, dtype=np.int64)
    ar64 = np.arange(64)
    d = np.arange(192)
    xm = 3 * (d % 64) + d // 64          # x-major block -> (c,x) interleaved
    refcol[0:192] = 576 + xm             # g5 -> tp_v2 block
    refcol[192:256] = 64 + ar64          # g1 -> tp_s1
    refcol[256:320] = 0 + ar64           # g0 -> s_e
    refcol[320:384] = 128 + ar64         # g2 -> tp_s2
    refcol[384:576] = 192 + xm           # g3 -> v_e block
    refcol[576:768] = 384 + xm           # g4 -> tp_v1 block
    return refcol


# ---------------------------------------------------------------- partition
def _partition_nodes(receivers: np.ndarray):
    """Assign each node to one of 128 bins (128 nodes per bin), balancing
    in-degree.  Returns (assign[node]->bin, pos[node]->0..127, max_load)."""
    deg = np.bincount(receivers, minlength=N_NODES).astype(np.int64)
    order = np.argsort(-deg, kind="stable")

    loads = np.zeros(N_BINS, dtype=np.int64)
    counts = np.zeros(N_BINS, dtype=np.int64)
    assign = np.empty(N_NODES, dtype=np.int64)
    heap = [(0, b) for b in range(N_BINS)]
    heapq.heapify(heap)
    for nd in order:
        while True:
            load, b = heapq.heappop(heap)
            if counts[b] < BIN_NODES and load == loads[b]:
                break
        assign[nd] = b
        counts[b] += 1
        loads[b] += deg[nd]
        if counts[b] < BIN_NODES:
            heapq.heappush(heap, (int(loads[b]), b))

    # repair pass: pairwise swaps toward exactly TARGET_LOAD per bin
    bin_nodes = [list(np.where(assign == b)[0]) for b in range(N_BINS)]
    for _ in range(20000):
        o = int(np.argmax(loads))
        u = int(np.argmin(loads))
        if loads[o] == TARGET_LOAD and loads[u] == TARGET_LOAD:
            break
        need = min(loads[o] - TARGET_LOAD, TARGET_LOAD - loads[u])
        if need <= 0:
            break
        degs_u = {}
        for nd in bin_nodes[u]:
            degs_u.setdefault(int(deg[nd]), nd)
        best = None
        for nd in bin_nodes[o]:
            da = int(deg[nd])
            for want in (da - need, da - need + 1, da - need - 1):
                if want >= 0 and want in degs_u and da - want > 0:
                    diff = abs(da - want - need)
                    if best is None or diff < best[0]:
                        best = (diff, nd, degs_u[want])
                    break
        if best is None:
            break
        _, a, bnode = best
        d = int(deg[a] - deg[bnode])
        bin_nodes[o].remove(a)
        bin_nodes[u].remove(bnode)
        bin_nodes[o].append(bnode)
        bin_nodes[u].append(a)
        assign[a], assign[bnode] = u, o
        loads[o] -= d
        loads[u] += d

    pos = np.empty(N_NODES, dtype=np.int64)
    for b in range(N_BINS):
        nds = np.where(assign == b)[0]
        pos[nds] = np.arange(len(nds))
    return assign, pos, int(loads.max())


# ---------------------------------------------------------------- program
_PROGRAM_CACHE = {}


def _mlp_groups(t_c):
    out = []
    j = 0
    while j < t_c:
        out.append((j, min(4, t_c - j)))
        j += 4
    return out


def _build_program(t_c: int):
    """Build the per-core Bass program (identical on all cores)."""
    S = N_CHUNKS * t_c * 128          # edge slots per core
    T = N_CHUNKS * t_c                # tiles per core
    rad_cols = ((N_CHUNKS + 2) // 3) * t_c * 128

    nc = bacc.Bacc()
    node_t = nc.declare_dram_parameter("node_t", [N_NODES, 256], BF16, isOutput=False)
    rad_h = nc.declare_dram_parameter("radial_s", [128, rad_cols], BF16, isOutput=False)
    attrs_h = nc.declare_dram_parameter("attrs", [128, T, 4], BF16, isOutput=False)
    sca_h = nc.declare_dram_parameter("sca", [128, T, 2], F32, isOutput=False)
    idx_h = nc.declare_dram_parameter("idx16", [128, S // 16], I16, isOutput=False)
    w1_h = nc.declare_dram_parameter("w1r", [128, 64], BF16, isOutput=False)
    w2_h = nc.declare_dram_parameter("w2s", [64, 64], BF16, isOutput=False)
    w3_h = nc.declare_dram_parameter("w3p", [64, 384], BF16, isOutput=False)
    out_h = nc.declare_dram_parameter("out", [N_CHUNKS * 128, 768], F32, isOutput=True)

    with tile.TileContext(nc) as tc:
        with (
            tc.tile_pool(name="const", bufs=1) as constp,
            tc.tile_pool(name="gat", bufs=3) as gatp,
            tc.tile_pool(name="acts", bufs=2) as actsp,
            tc.tile_pool(name="wslab", bufs=3) as wsp,
            tc.tile_pool(name="msg", bufs=3) as msgp,
            tc.tile_pool(name="oh", bufs=3) as ohp,
            tc.tile_pool(name="small", bufs=3) as smallp,
            tc.tile_pool(name="outs", bufs=2) as outsp,
            tc.tile_pool(name="pmlp", bufs=4, space="PSUM") as pmlp,
            tc.tile_pool(name="pw", bufs=2, space="PSUM") as pwp,
            tc.tile_pool(name="pacc", bufs=1, space="PSUM") as paccp,
        ):
            w1s = constp.tile([128, 64], BF16)
            w2s = constp.tile([64, 64], BF16)
            w3s = constp.tile([64, 384], BF16)
            attrs = constp.tile([128, T, 4], BF16)
            sca = constp.tile([128, T, 2], F32)
            idxs = constp.tile([128, S // 16], I16)
            rad = constp.tile([128, rad_cols], BF16)
            iota_b = constp.tile([128, 128], BF16)
            warm = constp.tile([128, 8], BF16)

            nc.gpsimd.dma_start(out=w1s[:], in_=w1_h[:])
            nc.gpsimd.dma_start(out=w2s[:], in_=w2_h[:])
            nc.gpsimd.dma_start(out=w3s[:], in_=w3_h[:])
            nc.gpsimd.dma_start(out=attrs[:], in_=attrs_h[:])
            nc.gpsimd.dma_start(out=sca[:], in_=sca_h[:])
            nc.gpsimd.dma_start(out=idxs[:], in_=idx_h[:])
            nc.gpsimd.dma_start(out=rad[:], in_=rad_h[:])
            nc.gpsimd.iota(iota_b[:], pattern=[[1, 128]], base=0,
                           channel_multiplier=0,
                           allow_small_or_imprecise_dtypes=True)
            # sem-warming: observe each preamble semaphore via a 1-wait op so
            # later consumers never need two fresh sem waits at once (the DVE
            # TT ISA slot only fits one).
            nc.vector.tensor_copy(warm[:, 0:1], iota_b[:, 0:1])
            nc.vector.tensor_copy(warm[:, 1:2], attrs[:, 0, 3:4])
            nc.vector.tensor_copy(warm[:, 2:3], rad[:, 0:1])
            nc.scalar.copy(warm[:, 3:4], sca[:, 0, 0:1])
            nc.scalar.copy(warm[:, 4:5], attrs[:, 0, 0:1])

            icols = t_c * 8  # idx columns per chunk
            for c in range(N_CHUNKS):
                gat = gatp.tile([128, t_c, 256], BF16)
                nc.gpsimd.dma_gather(
                    out_ap=gat[:],
                    in_ap=node_t[:],
                    idxs_ap=idxs[:, c * icols:(c + 1) * icols],
                    num_idxs=t_c * 128,
                    num_idxs_reg=t_c * 128,
                    elem_size=256,
                )

                acc = paccp.tile([128, 1024], F32)

                # ---- radial MLP (groups of <=4 tiles)
                pb = 32 * (c % 3)
                cb = (c // 3) * 1024
                h2_list = []
                for (sub, gsz) in _mlp_groups(t_c):
                    w_ = 128 * gsz
                    ph1 = pmlp.tile([64, 512], F32, tag="pmlp")
                    nc.tensor.matmul(
                        ph1[:, :w_], lhsT=w1s[pb:pb + 8, :],
                        rhs=rad[pb:pb + 8, cb + sub * 128:cb + sub * 128 + w_],
                        start=True, stop=True)
                    h1 = actsp.tile([64, 512], BF16, tag="h1")
                    nc.scalar.activation(h1[:, :w_], ph1[:, :w_], AF.Silu)
                    ph2 = pmlp.tile([64, 512], F32, tag="pmlp")
                    nc.tensor.matmul(
                        ph2[:, :w_], lhsT=w2s[:], rhs=h1[:, :w_],
                        start=True, stop=True)
                    h2 = actsp.tile([64, 512], BF16, tag="h2")
                    nc.scalar.activation(h2[:, :w_], ph2[:, :w_], AF.Silu)
                    h2_list.append((sub, gsz, h2))

                def h2_slice(j):
                    for sub, gsz, h2 in h2_list:
                        if sub <= j < sub + gsz:
                            return h2[:, (j - sub) * 128:(j - sub + 1) * 128]
                    raise AssertionError

                # ---- per-edge weights (mm3) into a chunk-wide w slab
                ws = wsp.tile([128, t_c, 384], BF16)
                for j in range(t_c):
                    pw = pwp.tile([128, 384], F32)
                    nc.tensor.matmul(
                        pw[:], lhsT=h2_slice(j), rhs=w3s[:],
                        start=True, stop=True)
                    nc.any.tensor_copy(out=ws[:, j, :], in_=pw[:])

                # ---- chunk-batched message construction (x-major vectors)
                s_ = gat[:, :, 0:64]
                v_ = gat[:, :, 64:256].rearrange("p j (x q) -> p j x q", q=64)
                wb = ws[:].rearrange("p j (b q) -> p j b q", q=64)
                evc = attrs[:, c * t_c:(c + 1) * t_c, 0:3]
                msgc = msgp.tile([128, t_c, 768], BF16)
                # g1|g0 <- s * (w1|w0): msg 64-blocks 3:5, w blocks 1:3
                nc.vector.tensor_tensor(
                    out=msgc[:].rearrange("p j (b q) -> p j b q", q=64)[:, :, 3:5, :],
                    in0=s_.unsqueeze(2).broadcast_to([128, t_c, 2, 64]),
                    in1=wb[:, :, 1:3, :], op=ALU.mult)
                # g5 <- v * w5 (cols 0:192); g3 <- v * w3 (cols 384:576)
                # split x{0,1} / x{2}: even mid-dim keeps the DVE 4x mode
                for (base, blk) in ((0, 0), (384, 4)):
                    nc.vector.tensor_tensor(
                        out=msgc[:, :, base:base + 128].rearrange(
                            "p j (x q) -> p j x q", q=64),
                        in0=v_[:, :, 0:2, :],
                        in1=wb[:, :, blk, :].unsqueeze(2).broadcast_to(
                            [128, t_c, 2, 64]),
                        op=ALU.mult)
                    nc.vector.tensor_tensor(
                        out=msgc[:, :, base + 128:base + 192],
                        in0=v_[:, :, 2, :],
                        in1=wb[:, :, blk, :], op=ALU.mult)
                # ev expanded (x-replicated) once per chunk on gpsimd
                evx = smallp.tile([128, t_c, 3, 64], BF16, tag="evx")
                nc.scalar.copy(
                    out=evx[:],
                    in_=evc.unsqueeze(3).broadcast_to([128, t_c, 3, 64]))
                # vv = v * ev ; tps2 = sum_x vv via two adds
                vv = smallp.tile([128, t_c, 3, 64], BF16, tag="vv")
                nc.vector.tensor_tensor(
                    out=vv[:, :, 0:2, :], in0=v_[:, :, 0:2, :],
                    in1=evx[:, :, 0:2, :], op=ALU.mult)
                nc.vector.tensor_tensor(
                    out=vv[:, :, 2, :], in0=v_[:, :, 2, :],
                    in1=evx[:, :, 2, :], op=ALU.mult)
                t01 = smallp.tile([128, t_c, 64], BF16, tag="t01")
                nc.vector.tensor_tensor(
                    out=t01[:], in0=vv[:, :, 0, :], in1=vv[:, :, 1, :],
                    op=ALU.add)
                tps2 = smallp.tile([128, t_c, 64], BF16, tag="tps2")
                nc.vector.tensor_tensor(
                    out=tps2[:], in0=t01[:], in1=vv[:, :, 2, :], op=ALU.add)
                # g2 <- tps2 * w2 (cols 320:384)
                nc.vector.tensor_tensor(
                    out=msgc[:, :, 320:384], in0=tps2[:],
                    in1=wb[:, :, 3, :], op=ALU.mult)
                # a4 = s * w4 ; g4 <- a4 x ev (cols 576:768)
                a4 = smallp.tile([128, t_c, 64], BF16, tag="a4")
                nc.vector.tensor_tensor(
                    out=a4[:], in0=s_, in1=wb[:, :, 5, :], op=ALU.mult)
                nc.vector.tensor_tensor(
                    out=msgc[:, :, 576:704].rearrange(
                        "p j (x q) -> p j x q", q=64),
                    in0=a4[:].unsqueeze(2).broadcast_to([128, t_c, 2, 64]),
                    in1=evx[:, :, 0:2, :], op=ALU.mult)
                nc.vector.tensor_tensor(
                    out=msgc[:, :, 704:768],
                    in0=a4[:], in1=evx[:, :, 2, :], op=ALU.mult)

                # ---- one-hots (chunk-batched) + scatter matmuls
                ohc = ohp.tile([128, t_c, 128], BF16, tag="oh")
                ohec = ohp.tile([128, t_c, 128], BF16, tag="ohe")
                for j in range(t_c):
                    t = c * t_c + j
                    nc.vector.tensor_scalar(
                        out=ohc[:, j, :], in0=iota_b[:],
                        scalar1=sca[:, t, 1:2], scalar2=None,
                        op0=ALU.is_equal)
                    nc.scalar.mul(ohec[:, j, :], ohc[:, j, :],
                                  sca[:, t, 0:1])
                    nc.tensor.matmul(
                        acc[:, 0:256], lhsT=ohec[:, j, :],
                        rhs=msgc[:, j, 0:256],
                        start=(j == 0), stop=(j == t_c - 1))
                    nc.tensor.matmul(
                        acc[:, 512:1024], lhsT=ohc[:, j, :],
                        rhs=msgc[:, j, 256:768],
                        start=(j == 0), stop=(j == t_c - 1))

                outs_t = outsp.tile([128, 768], F32)
                nc.any.tensor_copy(out=outs_t[:, 0:256], in_=acc[:, 0:256])
                nc.any.tensor_copy(out=outs_t[:, 256:768], in_=acc[:, 512:1024])
                nc.sync.dma_start(
                    out=out_h[c * 128:(c + 1) * 128, :], in_=outs_t[:])

    nc.compile()
    return nc


def _get_program(t_c: int):
    if t_c not in _PROGRAM_CACHE:
        _PROGRAM_CACHE[t_c] = _build_program(t_c)
    return _PROGRAM_CACHE[t_c]


# ---------------------------------------------------------------- host prep
def _prepare(inputs):
    node_feats = np.asarray(inputs["node_feats"], dtype=np.float32)
    edge_features = np.asarray(inputs["edge_features"], dtype=np.float32)
    radial = np.asarray(inputs["radial_embedding"], dtype=np.float32)
    w1 = np.asarray(inputs["w1"], dtype=np.float32)
    w2 = np.asarray(inputs["w2"], dtype=np.float32)
    w3 = np.asarray(inputs["w3"], dtype=np.float32)
    senders = np.asarray(inputs["senders"]).astype(np.int64)
    receivers = np.asarray(inputs["receivers"]).astype(np.int64)

    assign, pos, max_load = _partition_nodes(receivers)
    t_c = max(8, (max_load + 127) // 128)
    S = N_CHUNKS * t_c * 128
    T = N_CHUNKS * t_c
    cap_cols = t_c * 128
    rad_cols = ((N_CHUNKS + 2) // 3) * cap_cols

    ebin = assign[receivers]                      # bin of each edge
    eord = np.argsort(ebin, kind="stable")        # edges grouped by bin
    counts = np.bincount(ebin, minlength=N_BINS)

    # slot table: per bin, edges at slots [bin_slot_base + 0 .. count)
    cap = t_c * 128
    slot_of_edge = np.empty(N_EDGES, dtype=np.int64)
    starts = np.concatenate([[0], np.cumsum(counts)])
    for b in range(N_BINS):
        es = eord[starts[b]:starts[b + 1]]
        slot_of_edge[es] = b * cap + np.arange(len(es))

    # per-slot edge data (global slot space: bin-major)
    S_all = N_BINS * cap
    sl_send = np.zeros(S_all, dtype=np.int16)
    sl_attr = np.zeros((S_all, 4), dtype=np.float32)
    sl_sca = np.zeros((S_all, 2), dtype=np.float32)
    sl_rad = np.zeros((S_all, N_RADIAL), dtype=np.float32)
    sl = slot_of_edge
    sl_send[sl] = senders.astype(np.int16)
    sl_sca[sl, 0] = edge_features[:, 0]
    sl_attr[sl, 0:3] = edge_features[:, 1:4]
    sl_sca[sl, 1] = pos[receivers].astype(np.float32)
    sl_attr[sl, 3] = pos[receivers].astype(np.float32)
    sl_rad[sl] = radial

    # weights (w1 replicated at the 4 quadrant partition bases)
    w1r = np.zeros((128, 64), dtype=np.float32)
    for b in range(3):
        w1r[32 * b:32 * b + N_RADIAL] = w1 / np.sqrt(N_RADIAL)
    w1r = w1r.astype(NP_BF16)
    w2s = (w2 / np.sqrt(HIDDEN)).astype(NP_BF16)
    w3p = _w3_permuted(w3).astype(NP_BF16)
    node_x = np.empty_like(node_feats)
    node_x[:, 0:64] = node_feats[:, 0:64]
    for x in range(3):
        node_x[:, 64 + 64 * x:128 + 64 * x] = node_feats[:, 64 + x::3]
    node_t = node_x.astype(NP_BF16)

    in_maps = []
    bin_rows = []  # node ids per core, in row order
    for k in range(N_CORES):
        lo, hi = k * N_CHUNKS * cap, (k + 1) * N_CHUNKS * cap
        send_k = sl_send[lo:hi]
        attr_k = sl_attr[lo:hi]
        sca_k = sl_sca[lo:hi]
        rad_k = sl_rad[lo:hi]

        idx16 = np.concatenate(
            [send_k[c * cap:(c + 1) * cap].reshape(-1, 16).T
             for c in range(N_CHUNKS)], axis=1)
        idx16 = np.tile(idx16, (8, 1))  # replicate across gpsimd cores
        attrs = attr_k.reshape(T, 128, 4).transpose(1, 0, 2).astype(NP_BF16)
        sca_a = sca_k.reshape(T, 128, 2).transpose(1, 0, 2).astype(np.float32)

        rad_s = np.zeros((128, rad_cols), dtype=NP_BF16)
        for c in range(N_CHUNKS):
            pb, cb = 32 * (c % 3), (c // 3) * cap_cols
            blk = rad_k[c * cap:(c + 1) * cap].T.astype(NP_BF16)
            rad_s[pb:pb + 8, cb:cb + cap] = blk

        in_maps.append({
            "node_t": node_t,
            "radial_s": rad_s,
            "attrs": np.ascontiguousarray(attrs),
            "sca": np.ascontiguousarray(sca_a),
            "idx16": np.ascontiguousarray(idx16),
            "w1r": w1r,
            "w2s": w2s,
            "w3p": w3p,
        })
        rows = []
        for c in range(N_CHUNKS):
            b = k * N_CHUNKS + c
            nds = np.where(assign == b)[0]
            rows.append(nds[np.argsort(pos[nds])])
        bin_rows.append(np.concatenate(rows))

    return t_c, in_maps, bin_rows


def _assemble(results, bin_rows):
    refcol = _ref_colmap()
    out = np.empty((N_NODES, 768), dtype=np.float32)
    for k in range(N_CORES):
        dev = results[k]["out"]
        out[bin_rows[k][:, None], refcol[None, :]] = dev
    return out


def kernel(**inputs):
    t_c, in_maps, bin_rows = _prepare(inputs)
    nc = _get_program(t_c)
    res = run_bass_kernel_spmd(nc, in_maps, list(range(N_CORES)))
    return _assemble(res.results, bin_rows)


def kernel_traced(**inputs):
    """Like kernel() but returns (output, BassKernelResults) with trace."""
    t_c, in_maps, bin_rows = _prepare(inputs)
    nc = _get_program(t_c)
    res = run_bass_kernel_spmd(nc, in_maps, list(range(N_CORES)), trace=True)
    return _assemble(res.results, bin_rows), res
